# revision 10
# baseline (speedup 1.0000x reference)
"""Trainium2 Bass kernel for nn_NeuralVMLayer (baked SwiGLU micro-FFN chain +
gated rank-1-logit attention), data-parallel over batch across 8 NeuronCores.

Contract: kernel(**inputs) takes FULL unsharded inputs (x [16,2048,192] plus
the tiny baked weights) and returns the FULL [16,2048,192] output.

Algorithm (derived host-side from the weight structure, exact math):
  1) FFN chain: x' = x + sum_l E_l @ (silu(Wu_l x + bu) * (Wg_l x + bg)) Wd_l.
     Only ~41 input channels are ever read and ~22 written; we matmul against a
     channel-compressed transpose of x (xSel) and patch inter-layer
     dependencies with host-precomputed cross-term matrices, so xSel never
     needs updating on device.
  2) Attention: both heads have rank-1 logits q.k = f(s)*g(t) with
     f = x.wq_eff*scale, g = x[.,POS].  softmax(f g)V is evaluated with a
     shifted Taylor/moment expansion:
       num_i(s)/den(s), den = sum_k (P_k/k!) a^k, num = sum_k (Q_ki/k!) a^k,
       a = f - c >= 0.35,  P_k = sum_t g^k e^{c g},  Q_ki = sum_t g^k e^{c g} v_i
     which is O(S*K) instead of O(S^2), K ~ 26 terms (error < 1e-8).
"""

import math

import numpy as np

# ---- problem constants (hardcoded per contract) ----
B, S, D = 16, 2048, 192
NCORES = 8
BPC = B // NCORES      # batch elems per core
PJ = S // 128          # 16 token chunks of 128
NFFN, HMAX = 7, 16
MEM_READ, MEM_WRITE, MEM_READY = 156, 157, 158
ALPHA_MARGIN = 0.35

_CACHE = {}


# --------------------------------------------------------------------------
# host-side weight preprocessing
# --------------------------------------------------------------------------

def _runs(idx_list):
    """split sorted int list into (start, width) runs"""
    runs = []
    for c in idx_list:
        if runs and c == runs[-1][0] + runs[-1][1]:
            runs[-1][1] += 1
        else:
            runs.append([c, 1])
    return [(a, b) for a, b in runs]


def _preprocess(Wu, bu, Wg, bg, Wd, Wq, Wk, Wv, Wo):
    meta = {}
    # effective hidden size per layer: units whose Wd column is nonzero
    h_eff = []
    for l in range(NFFN):
        nz = np.nonzero(np.any(Wd[l] != 0, axis=0))[0]
        h_eff.append(int(nz.max()) + 1 if len(nz) else 1)
    meta["h_eff"] = h_eff
    # FFN read/write channel sets
    ffn_in = set()
    for l in range(NFFN):
        ffn_in |= set(np.nonzero(np.any(Wu[l] != 0, axis=0))[0].tolist())
        ffn_in |= set(np.nonzero(np.any(Wg[l] != 0, axis=0))[0].tolist())
    out_ch = [sorted(np.nonzero(np.any(Wd[l] != 0, axis=1))[0].tolist())
              for l in range(NFFN)]
    ffn_out = set().union(*[set(o) for o in out_ch])

    # attention rank-1 decomposition (per head)
    wq_eff, wdirs, wscale = [], [], []
    scale = 1.0 / math.sqrt(Wq.shape[1])
    for a in range(2):
        K = Wk[a]
        norms = np.linalg.norm(K, axis=1)
        i0 = int(np.argmax(norms))
        wdir = K[i0] / norms[i0]
        s = K @ wdir
        assert np.linalg.norm(K - np.outer(s, wdir)) < 1e-5 * max(1.0, norms.max()), \
            "attention Wk is not rank-1; structured kernel inapplicable"
        wq_eff.append((s @ Wq[a]) * scale)
        # g channel: require 1-sparse direction
        nzw = np.nonzero(wdir)[0]
        assert len(nzw) == 1, "Wk direction not 1-sparse"
        wdirs.append(int(nzw[0]))
        wscale.append(float(wdir[nzw[0]]))
    assert wdirs[0] == wdirs[1] and abs(wscale[0] - wscale[1]) < 1e-7, \
        "heads use different g directions"
    meta["cg"] = wdirs[0]
    meta["gs"] = wscale[0]

    # Wo-folded value projections; v-output channels per head
    vch, Wvo = [], []
    for a in range(2):
        ch = sorted(np.nonzero(np.any(Wo[a] != 0, axis=1))[0].tolist())
        vch.append(ch)
        Wvo.append(np.einsum('ci,id->dc', Wo[a][ch, :], Wv[a]))  # [D, nc]
    assert len(vch[0]) + len(vch[1]) == 9, f"unexpected v-channel count {vch}"
    meta["vch"] = vch

    # attention reads must be disjoint from FFN writes (else xSel staleness
    # and the pre-FFN g/v reads would be wrong)
    attn_read = set([meta["cg"]])
    for a in range(2):
        attn_read |= set(np.nonzero(wq_eff[a])[0].tolist())
        attn_read |= set(np.nonzero(np.any(Wvo[a] != 0, axis=1))[0].tolist())
    assert not (attn_read & ffn_out), (attn_read, ffn_out)
    assert not ({MEM_READ, MEM_WRITE} & ffn_out)

    # channel selection for the channel-major compressed transpose
    sel = sorted(ffn_in | attn_read - {meta["cg"]} |
                 set(np.nonzero(wq_eff[0])[0].tolist()) |
                 set(np.nonzero(wq_eff[1])[0].tolist()))
    nsel = len(sel)
    chrow = {c: r for r, c in enumerate(sel)}
    meta["sel"] = sel
    meta["nsel"] = nsel
    meta["sel_runs"] = _runs(sel)

    # ---- cW blob [NSEL, *]: channel-major lhsT weights ----
    cw_cols = []
    cw_off = {}

    def cw_add(name, mat):  # mat [NSEL, w]
        cw_off[name] = sum(c.shape[1] for c in cw_cols)
        cw_cols.append(np.ascontiguousarray(mat, np.float32))

    for l in range(NFFN):
        m = np.zeros((nsel, 48), np.float32)
        for r, c in enumerate(sel):
            m[r, :HMAX] = Wu[l, :, c]
            m[r, 32:48] = Wg[l, :, c]
        cw_add(f"ug{l}", m)
    fq = np.zeros((nsel, 16), np.float32)
    for a in range(2):
        for c in np.nonzero(wq_eff[a])[0]:
            fq[chrow[c], a] = wq_eff[a][c]
    cw_add("fq", fq)
    vv = np.zeros((nsel, 9), np.float32)
    col = 0
    v_col = []
    for a in range(2):
        v_col.append(col)
        for j, c in enumerate(vch[a]):
            vv[:, col + j] = Wvo[a][sel, j]
        col += len(vch[a])
    cw_add("vv", vv)
    meta["v_col"] = v_col
    cW = np.concatenate(cw_cols, axis=1)
    meta["cw_off"] = cw_off
    meta["cw_w"] = cW.shape[1]

    # ---- cH blob [HMAX, *]: hid-partition lhsT weights / bias columns ----
    ch_cols = []
    ch_off = {}

    def ch_add(name, mat):  # mat [HMAX, w]
        ch_off[name] = sum(c.shape[1] for c in ch_cols)
        ch_cols.append(np.ascontiguousarray(mat, np.float32))

    cross = []  # (l, m) pairs with nonzero coupling
    for l in range(NFFN):
        h = h_eff[l]
        # bias columns
        bc = np.zeros((HMAX, 2), np.float32)
        bc[:, 0] = bu[l]
        bc[:, 1] = bg[l]
        ch_add(f"b{l}", bc)
        # down lhs-rhs: rhs[i, c] = Wd[l, out_ch[c], i]
        nc_l = len(out_ch[l])
        dm = np.zeros((HMAX, nc_l), np.float32)
        for cix, c in enumerate(out_ch[l]):
            dm[:, cix] = Wd[l, c, :]
        ch_add(f"wd{l}", dm)
        # cross terms vs earlier layers
        for mprev in range(l):
            hm = h_eff[mprev]
            Om = out_ch[mprev]
            Cm = np.zeros((HMAX, 48), np.float32)
            for k in range(HMAX):
                for i in range(HMAX):
                    Cm[k, i] = sum(Wu[l, i, c] * Wd[mprev, c, k] for c in Om)
                    Cm[k, 32 + i] = sum(Wg[l, i, c] * Wd[mprev, c, k] for c in Om)
            if np.any(Cm != 0):
                cross.append((l, mprev))
                ch_add(f"c{l}_{mprev}", Cm)
    cH = np.concatenate(ch_cols, axis=1)
    meta["ch_off"] = ch_off
    meta["ch_w"] = cH.shape[1]
    meta["cross"] = cross
    meta["out_ch"] = out_ch

    # delta-psum column layout + add-runs
    pcol = 0
    ffn_runs = []  # (x_channel_start, width, psum_col_start)
    dcol_off = []
    for l in range(NFFN):
        dcol_off.append(pcol)
        for (c0, w) in _runs(out_ch[l]):
            ffn_runs.append((c0, w, pcol + out_ch[l].index(c0)))
        pcol += len(out_ch[l])
    meta["dcol_off"] = dcol_off
    meta["ffn_runs"] = ffn_runs
    meta["dcols"] = pcol

    # attn update runs: U rows are [M0_0, U(vch0), M0_1, U(vch1)]
    # U row layout: [M0_0, U0(vch0), M0_1, U1(vch1)]
    m0row = [0, 1 + len(vch[0])]
    urow = [1, 2 + len(vch[0])]
    attn_runs = []
    for a in range(2):
        for (c0, w) in _runs(vch[a]):
            attn_runs.append((a, c0, w, urow[a] + vch[a].index(c0)))
    meta["m0row"] = m0row
    meta["attn_runs"] = attn_runs

    # Taylor shift + term count from weight-derived bounds (x in [0,1])
    cshift, KT = [], 8
    gmax = max(meta["gs"], 0.0)  # g = gs * x_cg, x in [0,1]
    for a in range(2):
        fmin = float(np.minimum(wq_eff[a], 0.0).sum())
        fmax = float(np.maximum(wq_eff[a], 0.0).sum())
        c = fmin - ALPHA_MARGIN
        cshift.append(c)
        z = (fmax - c) * gmax
        k = 8
        while z ** (k + 1) / math.factorial(k + 1) > 1e-8:
            k += 1
        KT = max(KT, k + 1)
    meta["cshift"] = cshift
    meta["KT"] = KT

    # ---- cP blob rows (broadcast to 128 partitions): crow, e0row ----
    crow = np.zeros(KT, np.float32)
    crow[1:] = meta["gs"]          # G_k = g^k with g = gs * x_cg
    e0row = np.zeros(KT, np.float32)
    e0row[0] = 1.0
    # ---- cK blob [KT, 4]: iota, invfact, negc (rows 0-1), pad ----
    cK = np.zeros((KT, 4), np.float32)
    cK[:, 0] = np.arange(KT)
    cK[:, 1] = [1.0 / math.factorial(k) for k in range(KT)]
    cK[0, 2] = -cshift[0]
    cK[1, 2] = -cshift[1]

    ident = np.eye(128, dtype=np.float32)

    # pack the whole constant blob (1-D f32)
    blob = []
    blob_off = {}

    def b_add(name, arr):
        blob_off[name] = sum(a.size for a in blob)
        blob.append(np.ascontiguousarray(arr, np.float32).ravel())

    b_add("cW", cW)
    b_add("cH", cH)
    b_add("cK", cK)
    b_add("crow", crow)
    b_add("e0row", e0row)
    b_add("ident", ident)
    wblob = np.concatenate(blob)
    meta["blob_off"] = blob_off
    meta["wblob"] = wblob
    return meta


# --------------------------------------------------------------------------
# device kernel
# --------------------------------------------------------------------------

def _build(meta):
    import concourse.bacc as bacc
    import concourse.bass as bass
    import concourse.mybir as mybir
    from concourse.tile import TileContext

    dt = mybir.dt
    F32, F32R = dt.float32, dt.float32r
    AF = mybir.ActivationFunctionType
    OP = mybir.AluOpType

    nsel = meta["nsel"]
    KT = meta["KT"]
    h_eff = meta["h_eff"]
    cwo, cho = meta["cw_off"], meta["ch_off"]
    bo = meta["blob_off"]
    dcols = meta["dcols"]
    cg = meta["cg"]

    nc = bacc.Bacc(None, target_bir_lowering=False, debug=False)
    x_in = nc.declare_dram_parameter("x", [BPC, S, D], F32, isOutput=False)
    lscratch = nc.dram_tensor("lscratch", [BPC, 2, S], F32)
    wb = nc.declare_dram_parameter("wb", [int(meta["wblob"].size)], F32,
                                   isOutput=False)
    out = nc.declare_dram_parameter("out", [BPC, S, D], F32, isOutput=True)

    def dram_bcast(src_ap, parts):
        """replicate a DRAM row across `parts` partitions"""
        return bass.AP(tensor=src_ap.tensor, offset=src_ap.offset,
                       ap=[[0, parts]] + [list(p) for p in src_ap.ap])

    with TileContext(nc) as tc:
        with (
            tc.tile_pool(name="const", bufs=1) as constp,
            tc.tile_pool(name="xtok", bufs=3) as xtokp,
            tc.tile_pool(name="cm", bufs=1) as cmp_,
            tc.tile_pool(name="hid", bufs=4) as hidp,
            tc.tile_pool(name="att", bufs=1) as attp,
            tc.tile_pool(name="tok", bufs=2) as tokp,
            tc.tile_pool(name="psA", bufs=1, space="PSUM") as psA,
            tc.tile_pool(name="psD", bufs=1, space="PSUM") as psD,
            tc.tile_pool(name="psT", bufs=1, space="PSUM") as psT,
        ):
            # ---------- constants ----------
            cWf = constp.tile([nsel, meta["cw_w"]], F32, tag="cWf")
            nc.sync.dma_start(out=cWf, in_=wb[bo["cW"]:bo["cW"] + nsel * meta["cw_w"]]
                              .rearrange("(p f) -> p f", p=nsel))
            cW = constp.tile([nsel, meta["cw_w"]], F32R, tag="cWr")
            nc.vector.tensor_copy(cW, cWf)

            cHf = constp.tile([HMAX, meta["ch_w"]], F32, tag="cHf")
            nc.sync.dma_start(out=cHf, in_=wb[bo["cH"]:bo["cH"] + HMAX * meta["ch_w"]]
                              .rearrange("(p f) -> p f", p=HMAX))
            cH = constp.tile([HMAX, meta["ch_w"]], F32R, tag="cHr")
            nc.vector.tensor_copy(cH, cHf)

            cK = constp.tile([KT, 4], F32, tag="cK")
            nc.sync.dma_start(out=cK, in_=wb[bo["cK"]:bo["cK"] + 4 * KT]
                              .rearrange("(p f) -> p f", p=KT))
            crow = constp.tile([128, KT], F32, tag="crow")
            nc.sync.dma_start(out=crow, in_=dram_bcast(wb[bo["crow"]:bo["crow"] + KT], 128))
            e0row = constp.tile([128, KT], F32, tag="e0row")
            nc.sync.dma_start(out=e0row, in_=dram_bcast(wb[bo["e0row"]:bo["e0row"] + KT], 128))
            ident = constp.tile([128, 128], F32, tag="ident")
            nc.sync.dma_start(out=ident, in_=wb[bo["ident"]:bo["ident"] + 128 * 128]
                              .rearrange("(p f) -> p f", p=128))

            for b in range(BPC):
                # ---------- load ----------
                xt = xtokp.tile([128, PJ, D], F32, tag="xt")
                # DRAM view: token t = j*128 + p  ->  [p, j, d]
                xv = x_in[b].rearrange("(j p) d -> p j d", p=128)
                nc.sync.dma_start(out=xt, in_=xv)
                rgb = tokp.tile([128, 1], F32, tag="rgb")
                nc.sync.dma_start(out=rgb, in_=dram_bcast(x_in[b, 0, MEM_READ:MEM_READ + 1], 128))
                wgb = tokp.tile([128, 1], F32, tag="wgb")
                nc.sync.dma_start(out=wgb, in_=dram_bcast(x_in[b, 0, MEM_WRITE:MEM_WRITE + 1], 128))

                # ---------- xSel: gather + transpose to channel-major ----------
                gath = tokp.tile([128, PJ, nsel], F32, tag="gath")
                off = 0
                for (c0, w) in meta["sel_runs"]:
                    nc.vector.tensor_copy(gath[:, :, off:off + w], xt[:, :, c0:c0 + w])
                    off += w
                selps = psA.tile([nsel, S], F32, tag="bigA")
                for j in range(PJ):
                    nc.tensor.transpose(selps[:, j * 128:(j + 1) * 128],
                                        gath[:, j, :], ident)
                xSel = cmp_.tile([nsel, S], F32R, tag="xSel")
                nc.scalar.activation(xSel, selps, AF.Copy)

                # ---------- FFN chain ----------
                dps = psD.tile([128, PJ, dcols], F32, tag="dps")
                hids = []
                for l in range(NFFN):
                    ug = psA.tile([48, S], F32, tag="bigA")
                    o = cwo[f"ug{l}"]
                    crosses = [mm for (ll, mm) in meta["cross"] if ll == l]
                    for q in range(4):
                        sl = slice(q * 512, (q + 1) * 512)
                        nc.tensor.matmul(ug[:, sl], cW[:, o:o + 48], xSel[:, sl],
                                         start=True, stop=(not crosses))
                        for ci, mm in enumerate(crosses):
                            oc = cho[f"c{l}_{mm}"]
                            nc.tensor.matmul(ug[:, sl], cH[:, oc:oc + 48],
                                             hids[mm][:, sl], start=False,
                                             stop=(ci == len(crosses) - 1))
                    su = tokp.tile([HMAX, S], F32, tag="su")
                    nc.scalar.activation(su, ug[0:HMAX], AF.Silu,
                                         bias=cHf[:, cho[f"b{l}"]:cho[f"b{l}"] + 1])
                    hid = hidp.tile([HMAX, S], F32R, tag="hid")
                    nc.vector.scalar_tensor_tensor(
                        hid, ug[32:48], cHf[:, cho[f"b{l}"] + 1:cho[f"b{l}"] + 2],
                        su, OP.add, OP.mult)
                    hids.append(hid)
                    # down-projection, token-major (plain f32: tiny N)
                    od = cho[f"wd{l}"]
                    nc_l = len(meta["out_ch"][l])
                    d0 = meta["dcol_off"][l]
                    for j in range(PJ):
                        nc.tensor.matmul(dps[:, j, d0:d0 + nc_l],
                                         hid[:, j * 128:(j + 1) * 128].bitcast(F32),
                                         cHf[:, od:od + nc_l], start=True, stop=True)
                # apply FFN deltas to x_tok
                for (c0, w, p0) in meta["ffn_runs"]:
                    nc.vector.tensor_tensor(xt[:, :, c0:c0 + w], xt[:, :, c0:c0 + w],
                                            dps[:, :, p0:p0 + w], OP.add)

                # ---------- attention ----------
                # f (channel-major) and ln(alpha)
                fps = psA.tile([16, S], F32, tag="bigA")
                o = cwo["fq"]
                for q in range(4):
                    sl = slice(q * 512, (q + 1) * 512)
                    nc.tensor.matmul(fps[:, sl], cW[:, o:o + 16], xSel[:, sl],
                                     start=True, stop=True)
                Lr = attp.tile([2, S], F32, tag="Lr")
                nc.scalar.activation(Lr, fps[0:2], AF.Ln, bias=cK[0:2, 2:3])
                nc.sync.dma_start(out=lscratch[b], in_=Lr)

                # G powers + wgt + WV (token-major)
                G = tokp.tile([128, PJ, KT], F32R, tag="G")
                Dt = tokp.tile([128, KT], F32, tag="Dt")
                for j in range(PJ):
                    nc.vector.tensor_scalar(Dt, crow, xt[:, j, cg:cg + 1], None, OP.mult)
                    nc.vector.tensor_tensor_scan(G[:, j, :], Dt, e0row, 1.0,
                                                 OP.mult, OP.add)
                wgt0 = tokp.tile([128, PJ], F32, tag="wgt0")
                nc.scalar.activation(wgt0, xt[:, :, cg], AF.Exp,
                                     scale=float(meta["cshift"][0] * meta["gs"]))
                wgt1 = tokp.tile([128, PJ], F32, tag="wgt1")
                nc.scalar.activation(wgt1, xt[:, :, cg], AF.Exp,
                                     scale=float(meta["cshift"][1] * meta["gs"]))
                # v projection (token-major)
                vps = psT.tile([128, PJ, 9], F32, tag="vps")
                ov = cwo["vv"]
                for j in range(PJ):
                    nc.tensor.matmul(vps[:, j, :],
                                     xSel[:, j * 128:(j + 1) * 128].bitcast(F32),
                                     cWf[:, ov:ov + 9], start=True, stop=True)
                WV = tokp.tile([128, PJ, 11], F32R, tag="WV")
                nv0 = len(meta["vch"][0])
                nc.vector.tensor_copy(WV[:, :, 0], wgt0)
                for c in range(nv0):
                    nc.vector.tensor_tensor(WV[:, :, 1 + c], wgt0,
                                            vps[:, :, meta["v_col"][0] + c], OP.mult)
                nc.vector.tensor_copy(WV[:, :, 1 + nv0], wgt1)
                for c in range(len(meta["vch"][1])):
                    nc.vector.tensor_tensor(WV[:, :, 2 + nv0 + c], wgt1,
                                            vps[:, :, meta["v_col"][1] + c], OP.mult)

                # moments [KT, 11]
                mps = psT.tile([KT, 11], F32, tag="mps")
                for j in range(PJ):
                    nc.tensor.matmul(mps, G[:, j, :].bitcast(F32),
                                     WV[:, j, :].bitcast(F32),
                                     start=(j == 0), stop=(j == PJ - 1))
                moms = attp.tile([KT, 32], F32R, tag="moms")
                nc.vector.tensor_scalar(moms, cK[:, 0:1].to_broadcast([KT, 32]),
                                        0.0, None, OP.mult)
                n0 = 1 + len(meta["vch"][0])
                nc.vector.tensor_scalar(moms[:, 0:n0], mps[:, 0:n0], cK[:, 1:2],
                                        None, OP.mult)
                n1 = 1 + len(meta["vch"][1])
                nc.vector.tensor_scalar(moms[:, 16:16 + n1], mps[:, n0:n0 + n1],
                                        cK[:, 1:2], None, OP.mult)

                # A = alpha^k (channel-major), per head; then eval U = moms^T A
                USB0 = attp.tile([1 + len(meta["vch"][0]), S], F32, tag="USB0")
                USB1 = attp.tile([1 + len(meta["vch"][1]), S], F32, tag="USB1")
                USBs = [USB0, USB1]
                for a in range(2):
                    Lb = attp.tile([KT, S], F32, tag="Lb")
                    nc.sync.dma_start(out=Lb, in_=dram_bcast(lscratch[b, a], KT))
                    kL = attp.tile([KT, S], F32, tag="kL")
                    nc.vector.tensor_scalar(kL, Lb, cK[:, 0:1], None, OP.mult)
                    A = attp.tile([KT, S], F32R, tag="A")
                    nc.scalar.activation(A, kL, AF.Exp)
                    nrows = 1 + len(meta["vch"][a])
                    ups = psA.tile([16, S], F32, tag="bigA")
                    for q in range(4):
                        sl = slice(q * 512, (q + 1) * 512)
                        nc.tensor.matmul(ups[:, sl],
                                         moms[:, 16 * a:16 * (a + 1)], A[:, sl],
                                         start=True, stop=True)
                    nc.scalar.activation(USBs[a], ups[0:nrows], AF.Copy)
                # transpose U to token-major; Ut col layout [M0_0,U0...,M0_1,U1...]
                utp = psT.tile([128, PJ, 11], F32, tag="utp")
                for a in range(2):
                    r0 = meta["m0row"][a]
                    nrows = 1 + len(meta["vch"][a])
                    for j in range(PJ):
                        nc.tensor.transpose(utp[:, j, r0:r0 + nrows],
                                            USBs[a][:, j * 128:(j + 1) * 128],
                                            ident[:nrows, :nrows])
                Ut = tokp.tile([128, PJ, 11], F32, tag="Ut")
                nc.vector.tensor_copy(Ut, utp)

                # finish: r = U/M0 * flag, add into x_tok
                flags = [rgb, wgb]
                for a in range(2):
                    rrec = tokp.tile([128, PJ], F32, tag="rrec")
                    nc.vector.reciprocal(rrec, Ut[:, :, meta["m0row"][a]])
                    sflag = tokp.tile([128, PJ], F32, tag="sflag")
                    nc.vector.tensor_scalar(sflag, rrec, flags[a], None, OP.mult)
                    for (aa, c0, w, u0) in meta["attn_runs"]:
                        if aa != a:
                            continue
                        tmp = tokp.tile([128, PJ, w], F32, tag="tmp")
                        sfb = bass.AP(tensor=sflag.tensor, offset=sflag[:, :].offset,
                                      ap=[list(p) for p in sflag[:, :].ap] + [[0, w]])
                        nc.vector.tensor_tensor(tmp, Ut[:, :, u0:u0 + w], sfb, OP.mult)
                        nc.vector.tensor_tensor(xt[:, :, c0:c0 + w],
                                                xt[:, :, c0:c0 + w], tmp, OP.add)

                # flags out
                fsum = tokp.tile([128, 1], F32, tag="fsum")
                nc.vector.tensor_tensor(fsum, rgb, wgb, OP.add)
                nc.vector.memset(xt[:, :, MEM_READ:MEM_READ + 3], 0.0)
                nc.vector.tensor_scalar(xt[:, :, MEM_READY], xt[:, :, MEM_READY],
                                        fsum, None, OP.add)

                # ---------- store ----------
                ov_ = out[b].rearrange("(j p) d -> p j d", p=128)
                nc.sync.dma_start(out=ov_, in_=xt)

    nc.compile()
    return nc


# --------------------------------------------------------------------------
# public entry point
# --------------------------------------------------------------------------

def kernel(x, ffn_W_up, ffn_b_up, ffn_W_gate, ffn_b_gate, ffn_W_down,
           att_Wq, att_Wk, att_Wv, att_Wo):
    from concourse.bass_utils import run_bass_kernel_spmd

    args = [np.asarray(a, np.float32) for a in
            (ffn_W_up, ffn_b_up, ffn_W_gate, ffn_b_gate, ffn_W_down,
             att_Wq, att_Wk, att_Wv, att_Wo)]
    x = np.ascontiguousarray(np.asarray(x, np.float32))
    key = tuple(a.tobytes() for a in args)
    if key not in _CACHE:
        meta = _preprocess(*args)
        _CACHE[key] = (meta, _build(meta))
    meta, nc = _CACHE[key]

    in_maps = [{"x": x[c * BPC:(c + 1) * BPC], "wb": meta["wblob"]}
               for c in range(NCORES)]
    res = run_bass_kernel_spmd(nc, in_maps, list(range(NCORES)))
    return np.concatenate([res.results[c]["out"] for c in range(NCORES)], axis=0)
